# revision 9
# baseline (speedup 1.0000x reference)
"""Embedding-lookup dot product, v2: dma_gather with SIGNED 64K-row windows.

out[i] = dot(user_matrix[location[i,0], :], goods_matrix[:, location[i,1]])

HW-verified mechanism: dma_gather idxs are int16 consumed SIGNED
(IVP_MULUSAN_2X32 in the Q7 ucode sign-extends), so one instruction
addresses a 65536-row window around a base placed mid-window.  That halves
the window count vs the int15 baseline: each factor matrix pads to 8
super-windows x 65536 rows.

Core grid 4x2: core (i,j) owns u-supers {2i,2i+1} (u-quarter) and g-supers
{4j..4j+3} (g-half) -> every pair belongs to exactly one core; 2x4=8
supercells per core, capacity CAP slots (multiple of 128; data-dependent,
asserted).  8 gather instructions per core (4 U covering (uc, gc-pair),
4 G covering (gc, uc=0..1)), 2*CAP idxs each, round-robin over the 4 SWDGE
queues.  Instruction order interleaves U halves and G chunks so the DVE can
start multiplying after 3 desc-gens instead of 5 (Q7 desc-gen is serial).

Padding idxs are spread positive values (never negative: the Q7 trims a
TRAILING run of negative idxs, and real idxs are legitimately negative
here) -- CAP > max cell count guarantees each instruction ends on a pad.

The DVE multiplies G chunk g against a strided view of the U tile and
reduces over K into res[128, 8*CAPCOLS] fp32; host maps back by rank.
"""

from contextlib import ExitStack

import numpy as np

import concourse.bacc as bacc
import concourse.mybir as mybir
from concourse import bass
from concourse.library_config import mlp
from concourse.bass_utils import run_bass_kernel_spmd

N_CORES = 8
USER_NUM = 500000
GOODS_NUM = 500000
K = 128
BATCH = 16384
P = 128

SWIN = 65536          # signed-idx addressable rows per gather window
NSUP = 8              # super-windows per factor matrix
UG, GG = 2, 4         # u-supers and g-supers per core (4x2 grid)
CELLS = UG * GG       # 8 supercells per core

_CACHE = {}


def build_nc(cap):
    f32 = mybir.dt.float32
    f16 = mybir.dt.float16
    i16 = mybir.dt.int16

    capcols = cap // P
    nu = 2 * cap                 # idxs per instruction (U: 2 cells; G: 2 cells)
    icols = nu // 16             # wrap16 idx columns per instruction
    idxcols = 8 * icols

    orig_barrier = bass.Bass.all_engine_barrier
    bass.Bass.all_engine_barrier = lambda self, *, sem_only=False: None
    try:
        nc = bacc.Bacc(
            "TRN2",
            target_bir_lowering=False,
            debug=False,
            num_devices=N_CORES,
            enable_partition_id=False,
            monotonic_sem_count=0,
            num_swdge_queues=4,
        )
    finally:
        bass.Bass.all_engine_barrier = orig_barrier

    tab = nc.dram_tensor("tab", [(UG + GG) * SWIN, K], f16, kind="ExternalInput")
    loc = nc.dram_tensor("loc", [P, idxcols], i16, kind="ExternalInput")
    out = nc.dram_tensor("out", [P, CELLS * capcols], f32, kind="ExternalOutput")

    with (
        nc.Block(no_gpsimd_drain=True) as block,
        nc.sbuf_tensor("idx", [P, idxcols], i16) as idx,
        nc.sbuf_tensor("gatU", [P, CELLS * capcols, K], f16) as gatU,  # uc-major
        nc.sbuf_tensor("gatG", [P, CELLS * capcols, K], f16) as gatG,  # gc-major
        nc.sbuf_tensor("res", [P, CELLS * capcols], f32) as res,
        nc.semaphore("io") as io,
        nc.semaphore("msem") as msem,
        nc.semaphore("vsem") as vsem,
        ExitStack() as stack,
    ):
        usems = [stack.enter_context(nc.semaphore(f"u{h}")) for h in range(2)]  # noqa: ANT232
        gsems = [stack.enter_context(nc.semaphore(f"g{g}")) for g in range(GG)]  # noqa: ANT232

        @block.sync
        def _(sync):
            sync.dma_start(out=idx[:], in_=loc[:]).then_inc(io, 16)
            sync.wait_ge(vsem, GG)
            sync.dma_start(out=out[:], in_=res[:]).then_inc(io, 16)
            sync.wait_ge(io, 16)

        # instruction emission order: U(uc=0,h=0), U(1,0), G0, G1,
        #                             U(0,1), U(1,1), G2, G3
        # U(uc,h) covers U-tile cells (uc, 2h) and (uc, 2h+1);
        # G(g) covers G-tile cells (g, uc=0) and (g, uc=1).
        # idx tile column layout matches this emission order.
        def ubase(uc):
            return uc * SWIN + SWIN // 2

        def gbase(g):
            return (UG + g) * SWIN + SWIN // 2

        @block.gpsimd
        def _(gpsimd):
            # hoist the shared idx-count register: one MOVE, not eight
            nreg = gpsimd.to_reg(nu)
            gpsimd.wait_ge(io, 16)
            pos = 0
            qn = 0
            for h in range(2):
                for uc in range(UG):
                    gpsimd.dma_gather(
                        out_ap=gatU[:, (uc * GG + 2 * h) * capcols:
                                    (uc * GG + 2 * h + 2) * capcols],
                        in_ap=tab[ubase(uc):ubase(uc) + SWIN // 2],
                        idxs_ap=idx[:, pos:pos + icols],
                        num_idxs=nu,
                        num_idxs_reg=nreg,
                        elem_size=K,
                        single_packet=False,
                        queue_num=qn % 4,
                    ).then_inc(usems[h], 16)
                    pos += icols
                    qn += 1
                for g in (2 * h, 2 * h + 1):
                    gpsimd.dma_gather(
                        out_ap=gatG[:, (g * UG) * capcols:(g * UG + 2) * capcols],
                        in_ap=tab[gbase(g):gbase(g) + SWIN // 2],
                        idxs_ap=idx[:, pos:pos + icols],
                        num_idxs=nu,
                        num_idxs_reg=nreg,
                        elem_size=K,
                        single_packet=False,
                        queue_num=qn % 4,
                    ).then_inc(gsems[g], 16)
                    pos += icols
                    qn += 1

        @block.vector
        def _(vector):
            # U tile viewed [p, uc, gc, capcols, k]; chunk g uses plane gc=g
            uview = gatU[:].rearrange(
                "p (a b c) k -> p a b c k", a=UG, b=GG, c=capcols
            )
            gview = gatG[:].rearrange(
                "p (a b c) k -> p a b c k", a=GG, b=UG, c=capcols
            )
            resv = res[:].rearrange("p (a b c) -> p a b c", a=GG, b=UG, c=capcols)
            nv = 0
            for g in range(GG):
                vector.wait_ge(usems[g // 2], 32)
                vector.wait_ge(gsems[g], 16)
                vector.tensor_mul(
                    out=gview[:, g],
                    in0=gview[:, g],
                    in1=uview[:, :, g],
                ).then_inc(msem, 1)
                nv += 1
                vector.wait_ge(msem, nv)
                vector.tensor_reduce(
                    out=resv[:, g],
                    in_=gview[:, g],
                    axis=mybir.AxisListType.X,
                    op=mybir.AluOpType.add,
                ).then_inc(vsem, 1)

    nc.compile()
    return nc


def _get_nc(cap):
    if cap not in _CACHE:
        _CACHE[cap] = build_nc(cap)
    return _CACHE[cap]


def _wrap16(seg):
    """Pack a flat idx list into the [16, n/16] wrap (idx i at [i%16, i//16])."""
    return seg.reshape(-1, 16).T


def make_in_maps(user_matrix, goods_matrix, location):
    user = np.asarray(user_matrix)
    goods = np.asarray(goods_matrix)
    userP = np.zeros((NSUP * SWIN, K), np.float16)
    userP[:USER_NUM] = user.astype(np.float16)
    goodsP = np.zeros((NSUP * SWIN, K), np.float16)
    goodsP[:GOODS_NUM] = goods.T.astype(np.float16)

    loc = np.asarray(location).astype(np.int64)
    l0, l1 = loc[:, 0], loc[:, 1]
    su, sg = l0 >> 16, l1 >> 16                  # super-window 0..7
    core = (su >> 1) * 2 + (sg >> 2)             # 4x2 grid
    uc, gc = su & 1, sg & 3                      # local super in core
    cell = uc * GG + gc                          # uc-major cell id 0..7
    key = core * CELLS + cell
    order = np.argsort(key, kind="stable")
    ks = key[order]
    starts = np.searchsorted(ks, np.arange(N_CORES * CELLS))
    rank = np.empty(BATCH, np.int64)
    rank[order] = np.arange(BATCH) - starts[ks]
    counts = np.bincount(key, minlength=N_CORES * CELLS)
    cap = int(np.ceil((counts.max() + 2) / P) * P)
    cap = max(cap, 384)
    capcols = cap // P

    # signed window-local offsets
    offu = (l0 - (su * SWIN + SWIN // 2)).astype(np.int16)
    offg = (l1 - (sg * SWIN + SWIN // 2)).astype(np.int16)

    nu = 2 * cap
    icols = nu // 16
    # padding idxs: positive spread (never negative -> no trailing trim)
    padu = ((np.arange(CELLS * cap) * 5237) % 32000 + 1).astype(np.int16)

    in_maps = []
    for c in range(N_CORES):
        i, j = c >> 1, c & 1
        tab = np.concatenate(
            [userP[2 * i * SWIN:(2 * i + UG) * SWIN],
             goodsP[4 * j * SWIN:(4 * j + GG) * SWIN]], axis=0
        )
        sel = core == c
        ucc, gcc, rr = uc[sel], gc[sel], rank[sel]
        arrU = padu.copy()                       # uc-major slots
        arrU[(ucc * GG + gcc) * cap + rr] = offu[sel]
        arrG = padu.copy()                       # gc-major slots
        arrG[(gcc * UG + ucc) * cap + rr] = offg[sel]
        # emission order: U(0,0), U(1,0), G0, G1, U(0,1), U(1,1), G2, G3
        segs = []
        for h in range(2):
            for u in range(UG):
                segs.append(arrU[(u * GG + 2 * h) * cap:(u * GG + 2 * h + 2) * cap])
            for g in (2 * h, 2 * h + 1):
                segs.append(arrG[g * UG * cap:(g * UG + 2) * cap])
        tile16 = np.concatenate([_wrap16(s) for s in segs], axis=1)
        assert tile16.shape == (16, 8 * icols)
        in_maps.append({"tab": tab, "loc": np.tile(tile16, (8, 1))})

    meta = {"core": core, "uc": uc, "gc": gc, "rank": rank, "cap": cap,
            "capcols": capcols}
    return in_maps, meta


def unshard(results, meta):
    res_all = np.stack([results[c]["out"] for c in range(N_CORES)])
    cap = meta["cap"]
    # G-major: pair at res[core, rank%128, (gc*UG+uc)*capcols + rank//128]
    col = (meta["gc"] * UG + meta["uc"]) * cap // P + meta["rank"] // P
    return res_all[meta["core"], meta["rank"] % P, col].reshape(BATCH, 1).astype(
        np.float32
    )


def run(in_maps, meta=None, trace=False, **kwargs):
    cap = meta["cap"] if meta else 384
    nc = _get_nc(cap)
    return run_bass_kernel_spmd(
        nc, in_maps, core_ids=list(range(N_CORES)), trace=trace, **kwargs
    )


def kernel(user_matrix, goods_matrix, location):
    in_maps, meta = make_in_maps(user_matrix, goods_matrix, location)
    res = run(in_maps, meta)
    return unshard(res.results, meta)
